# revision 25
# baseline (speedup 1.0000x reference)
"""Autoformer attention block kernel for 8 TRN2 NeuronCores.

Math reduction (validated vs reference to 1.3e-7 in numpy):
 - output = x + AutoCorrelation(series_decomp(LN(x)))  (final decomp s2+t2 == x2)
 - mean over lags of the FFT cross-correlation == (sum_t Q)*(sum_t K)  (DC bin),
   so no FFT is needed: top-k stats come from column sums of `seasonal`
   (nonzero only on the 48 boundary rows via the phi trick).
 - the series-decomp band (I - MA25) and the 64-tap delay FIR commute with
   the d-axis projection, so they FUSE into one 88-tap circular FIR C applied
   to y = LN(x) @ Wvo, with a rank-24 boundary correction for the clamped
   moving-average windows:  out = x + C(y) + corr + (bv@Wo + bo).
 - Wvo = diag(gamma) @ Wv @ Wo; beta drops (C and corr have row-sum 0 paths).

Schedule: LN -> z -> (DRAM xbar DMA-transpose per 768-row quarter) ->
y = z @ Wvo all run before the collective result is needed; the
data-dependent tail is only the toeplitz build + 3 tap matmuls per tile.
The PE never transposes data tiles.

Sharding: data-parallel over batch (B=8 -> 8 cores); one [64]-float AllGather
for the global top-40 channel selection.
"""

import sys

if "/opt/trn_rl_repo" not in sys.path:
    sys.path.insert(0, "/opt/trn_rl_repo")

import numpy as np

L = 3072
D = 512
NT = 24   # 128-row time tiles
NP = 12   # 256-row tile pairs
NQ = 4    # 768-row quarters (3 pairs each)
H = 8
DK = 64
KTOP = 40
PAD = 12
EPS = 1e-5
NCORES = 8
HL = float(H * L)

_CACHE = {}


def _np_consts():
    t = np.arange(L)
    lo = np.maximum(t - PAD, 0)
    hi = np.minimum(t + PAD + 1, L)
    inv = 1.0 / (hi - lo).astype(np.float64)

    phi = np.ones(L, np.float64)
    for s in range(L):
        a = max(0, s - PAD)
        b = min(L, s + PAD + 1)
        phi[s] -= inv[a:b].sum()
    phi48 = np.zeros((128, 1), np.float32)
    phi48[:24, 0] = phi[:24]
    phi48[64:88, 0] = phi[-24:]

    ident = np.eye(128, dtype=np.float32)

    # g-map: g~[k] = w~[k-12] - (1/25) sum_{|e|<=12} w~[k-12-e], as lhsT [64, 88]
    MgT = np.zeros((64, 88), np.float64)
    for d in range(64):
        for k in range(88):
            delta = k - 12
            v = 1.0 if d == delta else 0.0
            if abs(delta - d) <= 12:
                v -= 1.0 / 25.0
            MgT[d, k] = v

    # boundary-correction E as lhsT [128, 24]: ey[u'] = sum_s E[s,u'] y[s]
    # (head y rows 0..23 at partitions 0..23, tail rows L-24..L-1 at 64..87)
    Econ = np.zeros((128, 24), np.float64)
    for up in range(24):
        u = up if up < 12 else L - 24 + up
        row = np.zeros(L)
        for e in range(-PAD, PAD + 1):
            row[(u + e) % L] += 1.0 / 25.0
        a = max(0, u - PAD)
        b = min(L, u + PAD + 1)
        row[a:b] -= 1.0 / (b - a)
        for s in np.nonzero(row)[0]:
            if s < 24:
                Econ[s, up] = row[s]
            elif s >= L - 24:
                Econ[64 + s - (L - 24), up] = row[s]
            else:
                raise AssertionError((up, s))

    rev = np.zeros((64, 64), np.float32)
    for i in range(64):
        rev[i, 63 - i] = 1.0
    return phi48, ident, MgT.astype(np.float32), Econ.astype(np.float32), rev


def _build():
    import concourse.bass as bass
    import concourse.tile as tile
    import concourse.mybir as mybir
    from concourse import bacc
    import bass_rust
    import ml_dtypes

    dt = mybir.dt
    f32 = dt.float32
    bf16 = dt.bfloat16
    AF = mybir.ActivationFunctionType
    ALU = mybir.AluOpType
    AX = mybir.AxisListType
    ts = bass.ts

    nc = bacc.Bacc(None, target_bir_lowering=False)

    xe = nc.dram_tensor("xb", [L, D], f32, kind="ExternalInput")
    wqe = nc.dram_tensor("Wq", [D, D], f32, kind="ExternalInput")
    wke = nc.dram_tensor("Wk", [D, D], f32, kind="ExternalInput")
    wve = nc.dram_tensor("Wv", [D, D], f32, kind="ExternalInput")
    woe = nc.dram_tensor("Wo", [D, D], f32, kind="ExternalInput")
    bqe = nc.dram_tensor("bq", [D], f32, kind="ExternalInput")
    bke = nc.dram_tensor("bk", [D], f32, kind="ExternalInput")
    bve = nc.dram_tensor("bv", [D], f32, kind="ExternalInput")
    boe = nc.dram_tensor("bo", [D], f32, kind="ExternalInput")
    gme = nc.dram_tensor("gamma", [D], f32, kind="ExternalInput")
    oute = nc.dram_tensor("out", [L, D], f32, kind="ExternalOutput")

    phi48, ident, MgT, Econ, rev = _np_consts()
    bf = ml_dtypes.bfloat16
    cphi = nc.inline_tensor(phi48.astype(bf), "c_phi")
    cid = nc.inline_tensor(ident.astype(bf), "c_id")
    cMgT = nc.inline_tensor(MgT.astype(bf), "c_MgT")
    cEcon = nc.inline_tensor(Econ.astype(bf), "c_Econ")
    crev = nc.inline_tensor(rev.astype(bf), "c_rev")
    cones = nc.inline_tensor(np.ones((1, 128), bf), "c_o128b")
    cones64 = nc.inline_tensor(np.ones((1, 64), np.float32), "c_o64")

    from contextlib import ExitStack

    with tile.TileContext(nc) as tc, ExitStack() as ctx:
        pc = ctx.enter_context(tc.tile_pool(name="consts", bufs=1))
        px = ctx.enter_context(tc.tile_pool(name="xpairs", bufs=NP))
        pzp = ctx.enter_context(tc.tile_pool(name="zpairs", bufs=3))
        pzT = ctx.enter_context(tc.tile_pool(name="zT", bufs=1))
        py = ctx.enter_context(tc.tile_pool(name="ytiles", bufs=NT))
        pwb = ctx.enter_context(tc.tile_pool(name="wbf", bufs=1))
        pst = ctx.enter_context(tc.tile_pool(name="stats", bufs=2))
        pbs = ctx.enter_context(tc.tile_pool(name="bnscr", bufs=2))
        psm = ctx.enter_context(tc.tile_pool(name="smalls", bufs=1))
        pout = ctx.enter_context(tc.tile_pool(name="osb", bufs=3))
        poutE = ctx.enter_context(tc.tile_pool(name="osbE", bufs=1))
        pdram = ctx.enter_context(tc.tile_pool(name="dram", bufs=1, space="DRAM"))
        q_y = ctx.enter_context(tc.tile_pool(name="ps_y", bufs=2, space="PSUM"))
        q_tp = ctx.enter_context(tc.tile_pool(name="ps_tp", bufs=1, space="PSUM"))
        q_big = ctx.enter_context(tc.tile_pool(name="ps_big", bufs=2, space="PSUM"))
        q_corr = ctx.enter_context(tc.tile_pool(name="ps_cr", bufs=1, space="PSUM"))
        q_sm = ctx.enter_context(tc.tile_pool(name="ps_sm", bufs=1, space="PSUM"))

        # ---------------- DRAM scratch ----------------
        canv_g = pdram.tile([128, 384], bf16, tag="canv_g")
        canv_c = pdram.tile([24, 192], bf16, tag="canv_c")
        g88d = pdram.tile([88], bf16, tag="g88d")
        wrevd = pdram.tile([64], bf16, tag="wrevd")
        ccin = pdram.tile([64], f32, tag="ccin")
        ccout = pdram.tile([8, 64], f32, tag="ccout")
        mvd = pdram.tile([64], f32, tag="mvd")
        zq = [pdram.tile([768, 512], bf16, tag=f"zq{q}", name=f"zq{q}") for q in range(NQ)]

        # ---------------- stats path (high priority) ----------------
        with tc.high_priority():
            x48 = pc.tile([128, 512], f32, tag="x48")
            nc.sync.dma_start(x48[0:24, :], xe[0:24, :])
            nc.sync.dma_start(x48[64:88, :], xe[L - 24 : L, :])
            phis = pc.tile([128, 1], bf16, tag="phis")
            nc.sync.dma_start(phis[:], cphi[:, :])
            idt = pc.tile([128, 128], bf16, tag="idt")
            nc.sync.dma_start(idt[:], cid[:, :])
            gammaP = pc.tile([128, 4], f32, tag="gammaP")
            nc.sync.dma_start(gammaP[:], gme[:].rearrange("(a b) -> b a", b=128))
            bqv = pc.tile([1, 512], f32, tag="bqv")
            nc.sync.dma_start(bqv[:], bqe[:])
            bkv = pc.tile([1, 512], f32, tag="bkv")
            nc.sync.dma_start(bkv[:], bke[:])

            # Wq/Wk interleaved bf16 via casting gpsimd DMAs:
            # wqk[:, c*1024 .. +512] = Wq chunk c; +512.. = Wk chunk c
            wqk = pwb.tile([128, 4096], bf16, tag="wqk")
            nc.gpsimd.dma_start(
                wqk[:].rearrange("p (c e d) -> p c e d", c=4, e=2)[:, :, 0, :],
                wqe[:, :].rearrange("(c p) d -> p c d", c=4),
            )
            nc.gpsimd.dma_start(
                wqk[:].rearrange("p (c e d) -> p c e d", c=4, e=2)[:, :, 1, :],
                wke[:, :].rearrange("(c p) d -> p c d", c=4),
            )

            # LN for the 48 boundary rows via bn_stats
            z48 = pc.tile([128, 512], bf16, tag="z48")
            nc.vector.memset(z48[:, :], 0.0)
            bs48 = pbs.tile([128, 8], f32, tag="bs48")
            nc.vector.bn_stats(bs48[0:88, 0:6], x48[0:88, :])
            nc.vector.bn_aggr(bs48[0:88, 6:8], bs48[0:88, 0:6])
            st8 = pst.tile([128, 4], f32, tag="st48")
            nc.vector.tensor_scalar(
                st8[0:88, 0:1], bs48[0:88, 7:8], 1.0, EPS, op0=ALU.mult, op1=ALU.add
            )
            nc.scalar.activation(st8[0:88, 1:2], st8[0:88, 0:1], AF.Sqrt)
            nc.vector.reciprocal(st8[0:88, 2:3], st8[0:88, 1:2])
            nc.vector.tensor_tensor(
                st8[0:88, 3:4], bs48[0:88, 6:7], st8[0:88, 2:3], op=ALU.mult
            )
            nc.vector.tensor_scalar(
                st8[0:88, 3:4], st8[0:88, 3:4], -1.0, None, op0=ALU.mult
            )
            nc.scalar.activation(
                z48[0:24, :], x48[0:24, :], AF.Identity,
                bias=st8[0:24, 3:4], scale=st8[0:24, 2:3],
            )
            nc.scalar.activation(
                z48[64:88, :], x48[64:88, :], AF.Identity,
                bias=st8[64:88, 3:4], scale=st8[64:88, 2:3],
            )
            qps = q_sm.tile([128, 8], f32, tag="sm")
            for c in range(4):
                nc.tensor.matmul(
                    qps[:, c : c + 1], z48[0:88, ts(c, 128)], phis[0:88, :],
                    start=True, stop=True,
                )

            bq_sc = pc.tile([1, 512], f32, tag="bq_sc")
            nc.scalar.mul(bq_sc[:], bqv[:], float(L))
            bk_sc = pc.tile([1, 512], f32, tag="bk_sc")
            nc.scalar.mul(bk_sc[:], bkv[:], float(L))

            qs_g = psm.tile([128, 4], bf16, tag="qsg")
            gb = psm.tile([128, 4], bf16, tag="gb")
            nc.vector.tensor_copy(gb[:], gammaP[:])
            nc.vector.tensor_tensor(qs_g[:], qps[:, 0:4], gb[:], op=ALU.mult)

            qq_ps = q_sm.tile([1, 512], f32, tag="sm")
            for c in range(4):
                nc.tensor.matmul(
                    qq_ps[:], qs_g[:, c : c + 1], wqk[:, c * 1024 : c * 1024 + 512],
                    start=(c == 0), stop=(c == 3),
                )
            qsv = psm.tile([1, 512], f32, tag="qsv")
            nc.vector.tensor_tensor(qsv[:], qq_ps[:], bq_sc[:], op=ALU.add)
            kk_ps = q_sm.tile([1, 512], f32, tag="sm")
            for c in range(4):
                nc.tensor.matmul(
                    kk_ps[:], qs_g[:, c : c + 1],
                    wqk[:, c * 1024 + 512 : (c + 1) * 1024],
                    start=(c == 0), stop=(c == 3),
                )
            ksv = psm.tile([1, 512], f32, tag="ksv")
            nc.vector.tensor_tensor(ksv[:], kk_ps[:], bk_sc[:], op=ALU.add)

            pr = psm.tile([1, 512], f32, tag="pr")
            nc.vector.tensor_tensor(pr[:], qsv[:], ksv[:], op=ALU.mult)
            mvr = psm.tile([1, 64], f32, tag="mvr")
            nc.vector.tensor_reduce(
                mvr[:], pr[:].rearrange("p (h c) -> p c h", h=H),
                axis=AX.X, op=ALU.add,
            )
            mv = psm.tile([1, 64], f32, tag="mv")
            nc.scalar.mul(mv[:], mvr[:], 1.0 / HL)
            nc.gpsimd.dma_start(ccin[:], mv[:])
            nc.gpsimd.dma_start(mvd[:], mv[:])

        # ---------------- weights (casting DMAs on gpsimd) ----------------
        wvb = pwb.tile([128, 2048], bf16, tag="wvb")
        nc.gpsimd.dma_start(
            wvb[:].rearrange("p (a d) -> p a d", a=4),
            wve[:, :].rearrange("(a p) d -> p a d", a=4),
        )
        wob = pwb.tile([128, 2048], bf16, tag="wob")
        nc.gpsimd.dma_start(
            wob[:].rearrange("p (a d) -> p a d", a=4),
            woe[:, :].rearrange("(a p) d -> p a d", a=4),
        )

        # zero toeplitz canvases
        zline = pc.tile([128, 384], bf16, tag="zline")
        nc.vector.memset(zline[:], 0.0)
        nc.gpsimd.dma_start(canv_g[:], zline[:])
        nc.gpsimd.dma_start(canv_c[:], zline[0:24, 0:192])

        # ---------------- x pair loads ----------------
        xp = [None] * NP

        def load_pair(k, eng):
            xt = px.tile([128, 1024], f32, tag="xp")
            eng.dma_start(
                xt[:].rearrange("p (c d) -> p c d", c=2),
                xe[256 * k : 256 * (k + 1), :].rearrange("(c p) d -> p c d", c=2),
            )
            xp[k] = xt

        for k in range(6):
            load_pair(k, nc.sync)
        for k in range(6, 12):
            load_pair(k, nc.scalar)

        bvP = pc.tile([128, 4], f32, tag="bvP")
        nc.sync.dma_start(bvP[:], bve[:].rearrange("(a b) -> b a", b=128))
        bov = pc.tile([1, 512], f32, tag="bov")
        nc.sync.dma_start(bov[:], boe[:])
        o128 = pc.tile([1, 128], bf16, tag="o128")
        nc.sync.dma_start(o128[:], cones[:, :])
        MgTsb = pc.tile([64, 88], bf16, tag="MgTsb")
        nc.sync.dma_start(MgTsb[:], cMgT[:, :])
        Econsb = pc.tile([128, 24], bf16, tag="Econsb")
        nc.sync.dma_start(Econsb[:], cEcon[:, :])
        revsb = pc.tile([64, 64], bf16, tag="revsb")
        nc.sync.dma_start(revsb[:], crev[:, :])
        o1x64 = pc.tile([1, 64], f32, tag="o1x64")
        nc.sync.dma_start(o1x64[:], cones64[:, :])

        # ---------------- weight prep: Wvo = diag(gamma) Wv Wo ----------------
        wvs = []
        for a in range(4):
            w = pwb.tile([128, 512], bf16, tag=f"wvs{a}")
            nc.scalar.activation(
                w[:], wvb[:, ts(a, 512)], AF.Identity, scale=gammaP[:, a : a + 1]
            )
            wvs.append(w)
        wvT = []
        for c in range(4):
            w = pwb.tile([128, 512], bf16, tag=f"wvT{c}")
            wvT.append(w)
        for a in range(4):
            tp = q_tp.tile([128, 512], bf16, tag="tp")
            for c in range(4):
                nc.tensor.transpose(tp[:, ts(c, 128)], wvs[a][:, ts(c, 128)], idt[:])
            for c in range(4):
                nc.vector.tensor_copy(wvT[c][:, ts(a, 128)], tp[:, ts(c, 128)])
        wvo = []
        for a in range(4):
            vps = q_y.tile([128, 512], f32, tag="y")
            for c in range(4):
                nc.tensor.matmul(
                    vps[:], wvT[c][:, ts(a, 128)], wob[:, ts(c, 512)],
                    start=(c == 0), stop=(c == 3),
                )
            w = pwb.tile([128, 512], bf16, tag=f"wvo{a}")
            nc.scalar.copy(w[:], vps[:])
            wvo.append(w)

        # cvb = bv @ Wo + bo, broadcast to [128, 512]
        bvPb = psm.tile([128, 4], bf16, tag="bvPb")
        nc.vector.tensor_copy(bvPb[:], bvP[:])
        cps = q_sm.tile([1, 512], f32, tag="sm")
        for c in range(4):
            nc.tensor.matmul(
                cps[:], bvPb[:, c : c + 1], wob[:, ts(c, 512)],
                start=(c == 0), stop=(c == 3),
            )
        cv_sb = psm.tile([1, 512], f32, tag="cv")
        nc.vector.tensor_tensor(cv_sb[:], cps[:], bov[:], op=ALU.add)
        cvb = psm.tile([1, 512], bf16, tag="cvb")
        nc.vector.tensor_copy(cvb[:], cv_sb[:])
        cbps = q_big.tile([128, 512], f32, tag="big")
        nc.tensor.matmul(cbps[:], o128[:], cvb[:], start=True, stop=True)
        cvbtile = pc.tile([128, 512], f32, tag="cvbtile")
        nc.vector.tensor_copy(cvbtile[:], cbps[:])

        # ---------------- main pipeline: LN -> z -> DRAM -> zT -> y ----------------
        ytiles = [None] * NT

        def emit_group(g):  # tiles 4g .. 4g+3 (pairs 2g, 2g+1)
            st = pst.tile([128, 16], f32, tag="st")
            for j in range(4):
                i = 4 * g + j
                xh = xp[i // 2][:, ts(i % 2, 512)]
                bs = pbs.tile([128, 8], f32, tag="bs")
                nc.vector.bn_stats(bs[:, 0:6], xh)
                nc.vector.bn_aggr(st[:, 2 * j : 2 * j + 2], bs[:, 0:6])
            nc.vector.tensor_scalar(
                st[:, 8:12], st[:, 1:8:2], 1.0, EPS, op0=ALU.mult, op1=ALU.add
            )
            nc.scalar.activation(st[:, 12:16], st[:, 8:12], AF.Sqrt)
            nc.vector.reciprocal(st[:, 8:12], st[:, 12:16])
            nc.vector.tensor_tensor(
                st[:, 12:16], st[:, 0:8:2], st[:, 8:12], op=ALU.mult
            )
            nc.vector.tensor_scalar(
                st[:, 12:16], st[:, 12:16], -1.0, None, op0=ALU.mult
            )
            for p in range(2):  # pairs 2g, 2g+1
                k = 2 * g + p
                zp = pzp.tile([128, 1024], bf16, tag="zp")
                for h in range(2):
                    j = 2 * p + h
                    nc.scalar.activation(
                        zp[:, ts(h, 512)], xp[k][:, ts(h, 512)], AF.Identity,
                        bias=st[:, 12 + j : 13 + j], scale=st[:, 8 + j : 9 + j],
                    )
                q, r = k // 3, k % 3
                zeng = nc.sync if q % 2 == 0 else nc.scalar
                zeng.dma_start(
                    zq[q][256 * r : 256 * r + 256, :].rearrange(
                        "(c p) d -> p c d", c=2
                    ),
                    zp[:].rearrange("p (c d) -> p c d", c=2),
                )

        zT = [[None] * 4 for _ in range(NQ)]

        def emit_ztrans(q):
            zeng = nc.sync if q % 2 == 0 else nc.scalar
            for c in range(4):
                t = pzT.tile([128, 768], bf16, tag=f"zT{q}{c}")
                zeng.dma_start(t[:], zq[q][:, ts(c, 128)], transpose=True)
                zT[q][c] = t

        def emit_y(i):
            q, j = i // 6, i % 6
            vps = q_y.tile([128, 512], f32, tag="y")
            for c in range(4):
                nc.tensor.matmul(
                    vps[:], zT[q][c][:, ts(j, 128)], wvo[c][:],
                    start=(c == 0), stop=(c == 3),
                )
            yv = py.tile([128, 512], bf16, tag="y")
            if i % 2 == 0:
                nc.scalar.copy(yv[:], vps[:])
            else:
                nc.vector.tensor_copy(yv[:], vps[:])
            ytiles[i] = yv

        emit_group(0)
        emit_group(1)          # pairs 0..3 done -> quarter 0 (pairs 0..2)
        emit_ztrans(0)
        for i in range(0, 6):
            emit_y(i)
        emit_group(2)          # pairs 4,5 -> quarter 1
        emit_ztrans(1)
        for i in range(6, 12):
            emit_y(i)
        emit_group(3)
        emit_group(4)          # pairs 6..9 -> quarter 2
        emit_ztrans(2)
        for i in range(12, 18):
            emit_y(i)
        emit_group(5)          # pairs 10,11 -> quarter 3
        emit_ztrans(3)
        for i in range(18, 24):
            emit_y(i)

        # collective launch: emitted after all pipeline DRAM-reading DMAs so
        # none of them pick up a false dependency on the AllGather; the gpsimd
        # queue still reaches this instruction within ~15us.
        nc.gpsimd.collective_compute(
            "AllGather",
            ALU.bypass,
            replica_groups=[list(range(NCORES))],
            ins=[ccin[:].opt()],
            outs=[ccout[:].opt()],
        )

        # residual with cvb folded in: xp[k] += cvbtile (both halves, vector)
        for k in range(NP):
            for h in range(2):
                nc.vector.tensor_tensor(
                    xp[k][:, ts(h, 512)], xp[k][:, ts(h, 512)], cvbtile[:],
                    op=ALU.add,
                )

        # ---------------- boundary ey = E(y) ----------------
        ytail = pc.tile([128, 512], bf16, tag="ytail")
        nc.gpsimd.dma_start(ytail[64:88, :], ytiles[23][104:128, :])
        eyps = q_big.tile([128, 512], f32, tag="big")
        nc.tensor.matmul(
            eyps[0:24, :], Econsb[0:24, :], ytiles[0][0:24, :],
            start=True, stop=False,
        )
        nc.tensor.matmul(
            eyps[0:24, :], Econsb[64:88, :], ytail[64:88, :],
            start=False, stop=True,
        )
        eysb = psm.tile([24, 512], bf16, tag="eysb")
        nc.vector.tensor_copy(eysb[:], eyps[0:24, :])

        # ---------------- collective readback -> weights -> toeplitz ----------------
        g8 = psm.tile([1, 512], f32, tag="g8")
        nc.sync.dma_start(g8[:], ccout[:])
        mvP = psm.tile([64, 1], f32, tag="mvP")
        nc.scalar.dma_start(mvP[:], mvd[:])
        sc = psm.tile([64, 8], f32, tag="scm")
        nc.scalar.activation(sc[:, 2:3], mvP[:], AF.Exp)  # local, pre-collective

        g_row = psm.tile([1, 64], f32, tag="grow")
        nc.vector.tensor_reduce(
            g_row[:], g8[:].rearrange("p (r c) -> p c r", r=NCORES),
            axis=AX.X, op=ALU.add,
        )
        ones64 = nc.const_aps.tensor(1.0, (64, 1))
        gP_ps = q_sm.tile([64, 1], f32, tag="sm")
        nc.tensor.matmul(
            gP_ps[:], g_row[:], nc.const_aps.tensor(1.0, (1, 1)),
            start=True, stop=True,
        )
        gP = psm.tile([64, 1], f32, tag="gP")
        nc.vector.tensor_copy(gP[:], gP_ps[:])
        gf_ps = q_sm.tile([64, 64], f32, tag="sm")
        nc.tensor.matmul(gf_ps[:], o1x64[:], g_row[:], start=True, stop=True)
        cmp = psm.tile([64, 64], f32, tag="cmp")
        nc.vector.tensor_tensor(
            cmp[:], gf_ps[:], gP[:].to_broadcast((64, 64)), op=ALU.is_gt
        )
        nc.vector.tensor_reduce(sc[:, 0:1], cmp[:], axis=AX.X, op=ALU.add)
        nc.vector.tensor_scalar(
            sc[:, 1:2], sc[:, 0:1], KTOP - 0.5, None, op0=ALU.is_lt
        )
        nc.vector.tensor_tensor(sc[:, 3:4], sc[:, 2:3], sc[:, 1:2], op=ALU.mult)
        s_ps = q_sm.tile([1, 1], f32, tag="sm")
        nc.tensor.matmul(s_ps[:], sc[:, 3:4], ones64, start=True, stop=True)
        rs = psm.tile([1, 1], f32, tag="rs")
        nc.vector.reciprocal(rs[:], s_ps[:])
        rsf_ps = q_sm.tile([64, 1], f32, tag="sm")
        nc.tensor.matmul(rsf_ps[:], o1x64[:], rs[:], start=True, stop=True)
        wf = psm.tile([64, 1], f32, tag="wf")
        nc.vector.tensor_tensor(wf[:], sc[:, 3:4], rsf_ps[:], op=ALU.mult)
        wfb = psm.tile([64, 1], bf16, tag="wfb")
        nc.vector.tensor_copy(wfb[:], wf[:])

        VP = bass_rust.VecI64Pair

        # g-kernel path (sync queue): g -> DRAM -> scatter -> canvas -> lhsTs
        g_ps = q_sm.tile([88, 1], f32, tag="sm")
        nc.tensor.matmul(g_ps[:], MgTsb[:], wfb[:], start=True, stop=True)
        g88b = psm.tile([88, 1], bf16, tag="g88b")
        nc.vector.tensor_copy(g88b[:], g_ps[:])
        nc.sync.dma_start(g88d[:], g88b[:])
        dstG = canv_g[:].flatten()
        dstG.ap = VP([[385, 128], [1, 88]])
        srcG = g88d[:].flatten()
        srcG.ap = VP([[0, 128], [1, 88]])
        nc.sync.dma_start(dstG, srcG)
        dstA = canv_g[0:1, 332:333].flatten()
        dstA.ap = VP([[385, 12], [1, 12]])
        srcA = g88d[:].flatten()
        srcA.ap = VP([[0, 12], [1, 12]])
        nc.sync.dma_start(dstA, srcA)
        canvsb = pc.tile([128, 384], bf16, tag="canvsb")
        nc.sync.dma_start(canvsb[:], canv_g[:])
        toep = {}
        for name, c0 in (("M", 12), ("N", 140), ("A", 216)):
            tpp = q_tp.tile([128, 512], bf16, tag="tp")
            nc.tensor.transpose(tpp[:, 0:128], canvsb[:, c0 : c0 + 128], idt[:])
            t_sb = pc.tile([128, 128], bf16, tag=f"toep{name}")
            nc.vector.tensor_copy(t_sb[:], tpp[:, 0:128])
            toep[name] = t_sb

        # corr path (scalar queue): wrev -> DRAM -> scatter -> corr lhsT
        wrev_ps = q_sm.tile([64, 1], f32, tag="sm")
        nc.tensor.matmul(wrev_ps[:], revsb[:], wfb[:], start=True, stop=True)
        wrevb = psm.tile([64, 1], bf16, tag="wrevb")
        nc.vector.tensor_copy(wrevb[:], wrev_ps[:])
        nc.scalar.dma_start(wrevd[:], wrevb[:])
        dstH = canv_c[0:1, 65:66].flatten()
        dstH.ap = VP([[193, 12], [1, 64]])
        srcH = wrevd[:].flatten()
        srcH.ap = VP([[0, 12], [1, 64]])
        nc.scalar.dma_start(dstH, srcH)
        dstT = canv_c[12:13, 53:54].flatten()
        dstT.ap = VP([[193, 12], [1, 64]])
        srcT = wrevd[:].flatten()
        srcT.ap = VP([[0, 12], [1, 64]])
        nc.scalar.dma_start(dstT, srcT)
        cvsb = pc.tile([24, 192], bf16, tag="cvsb")
        nc.scalar.dma_start(cvsb[:], canv_c[:])
        nc.vector.tensor_tensor(
            cvsb[:, 0:64], cvsb[:, 0:64], cvsb[:, 128:192], op=ALU.add
        )
        corrP = q_corr.tile([128, 512], f32, tag="cr")
        nc.tensor.matmul(
            corrP[:], cvsb[:, 0:128], eysb[:], start=True, stop=True
        )

        # ---------------- taps + residual + output ----------------
        # order: 1..22 first (corr-free), then 0 and 23 (need corrP)
        obs = {}

        def emit_tap(i):
            tps = q_big.tile([128, 512], f32, tag="big")
            nc.tensor.matmul(
                tps[:], toep["A"][:], ytiles[(i + NT - 1) % NT][:],
                start=True, stop=False,
            )
            nc.tensor.matmul(
                tps[:], toep["M"][:], ytiles[i][:], start=False, stop=False
            )
            nc.tensor.matmul(
                tps[:], toep["N"][:], ytiles[(i + 1) % NT][:],
                start=False, stop=True,
            )
            k, hh = i // 2, i % 2
            if k not in obs:
                if k in (0, NP - 1):
                    obs[k] = poutE.tile([128, 1024], f32, tag=f"obE{k}", name="obE")
                else:
                    obs[k] = pout.tile([128, 1024], f32, tag="ob", name="ob")
            ob = obs[k]
            nc.vector.tensor_tensor(
                ob[:, ts(hh, 512)], xp[k][:, ts(hh, 512)], tps[:], op=ALU.add
            )
            if i == 0:
                nc.vector.tensor_tensor(
                    ob[0:32, 0:512], ob[0:32, 0:512], corrP[0:32, :], op=ALU.add
                )
            if i == 23:
                nc.vector.tensor_tensor(
                    ob[32:64, 512:1024], ob[32:64, 512:1024], corrP[32:64, :],
                    op=ALU.add,
                )
                nc.vector.tensor_tensor(
                    ob[64:128, 512:1024], ob[64:128, 512:1024], corrP[64:128, :],
                    op=ALU.add,
                )

        def flush_pair(k, eng):
            eng.dma_start(
                oute[256 * k : 256 * (k + 1), :].rearrange(
                    "(c p) d -> p c d", c=2
                ),
                obs[k][:].rearrange("p (c d) -> p c d", c=2),
            )

        for i in range(1, 23):
            emit_tap(i)
            if i % 2 == 1 and i >= 3:  # pair i//2 in 1..10 complete
                flush_pair(i // 2, nc.gpsimd)
        emit_tap(0)
        emit_tap(23)
        flush_pair(0, nc.gpsimd)
        flush_pair(11, nc.gpsimd)

    nc.finalize()
    return nc


def _get_nc():
    if "nc" not in _CACHE:
        _CACHE["nc"] = _build()
    return _CACHE["nc"]


def kernel_ext(inputs, trace=False):
    from concourse.bass_utils import run_bass_kernel_spmd

    nc = _get_nc()
    x = np.ascontiguousarray(inputs["x"], np.float32)
    common = {
        k: np.ascontiguousarray(inputs[k], np.float32)
        for k in ["Wq", "Wk", "Wv", "Wo", "bq", "bk", "bv", "bo", "gamma"]
    }
    in_maps = [{"xb": x[i], **common} for i in range(NCORES)]
    res = run_bass_kernel_spmd(nc, in_maps, list(range(NCORES)), trace=trace)
    out = np.stack([res.results[i]["out"] for i in range(NCORES)], axis=0)
    return out, res


def kernel(**inputs):
    out, _ = kernel_ext(inputs)
    return out


# revision 26
# speedup vs baseline: 1.3031x; 1.3031x over previous
"""Autoformer attention block kernel for 8 TRN2 NeuronCores.

Math reduction (validated vs reference to 1.3e-7 in numpy):
 - output = x + AutoCorrelation(series_decomp(LN(x)))  (final decomp s2+t2 == x2)
 - mean over lags of the FFT cross-correlation == (sum_t Q)*(sum_t K)  (DC bin),
   so no FFT is needed: top-k stats come from column sums of `seasonal`
   (nonzero only on the 48 boundary rows via the phi trick).
 - the series-decomp band (I - MA25) and the 64-tap delay FIR commute with
   the d-axis projection, so they FUSE into one 88-tap circular FIR C applied
   to y = LN(x) @ Wvo, with a rank-24 boundary correction for the clamped
   moving-average windows:  out = x + C(y) + corr + (bv@Wo + bo).
 - Wvo = diag(gamma) @ Wv @ Wo; beta drops (C and corr have row-sum 0 paths).

Schedule: LN -> z -> (DRAM xbar DMA-transpose per 768-row quarter) ->
y = z @ Wvo all run before the collective result is needed; the
data-dependent tail is only the toeplitz build + 3 tap matmuls per tile.
The PE never transposes data tiles.

Sharding: data-parallel over batch (B=8 -> 8 cores); one [64]-float AllGather
for the global top-40 channel selection.
"""

import sys

if "/opt/trn_rl_repo" not in sys.path:
    sys.path.insert(0, "/opt/trn_rl_repo")

import numpy as np

L = 3072
D = 512
NT = 24   # 128-row time tiles
NP = 12   # 256-row tile pairs
NQ = 4    # 768-row quarters (3 pairs each)
H = 8
DK = 64
KTOP = 40
PAD = 12
EPS = 1e-5
NCORES = 8
HL = float(H * L)

_CACHE = {}


def _np_consts():
    t = np.arange(L)
    lo = np.maximum(t - PAD, 0)
    hi = np.minimum(t + PAD + 1, L)
    inv = 1.0 / (hi - lo).astype(np.float64)

    phi = np.ones(L, np.float64)
    for s in range(L):
        a = max(0, s - PAD)
        b = min(L, s + PAD + 1)
        phi[s] -= inv[a:b].sum()
    phi48 = np.zeros((128, 1), np.float32)
    phi48[:24, 0] = phi[:24]
    phi48[64:88, 0] = phi[-24:]

    ident = np.eye(128, dtype=np.float32)

    # g-map: g~[k] = w~[k-12] - (1/25) sum_{|e|<=12} w~[k-12-e], as lhsT [64, 88]
    MgT = np.zeros((64, 88), np.float64)
    for d in range(64):
        for k in range(88):
            delta = k - 12
            v = 1.0 if d == delta else 0.0
            if abs(delta - d) <= 12:
                v -= 1.0 / 25.0
            MgT[d, k] = v

    # boundary-correction E as lhsT [128, 24]: ey[u'] = sum_s E[s,u'] y[s]
    # (head y rows 0..23 at partitions 0..23, tail rows L-24..L-1 at 64..87)
    Econ = np.zeros((128, 24), np.float64)
    for up in range(24):
        u = up if up < 12 else L - 24 + up
        row = np.zeros(L)
        for e in range(-PAD, PAD + 1):
            row[(u + e) % L] += 1.0 / 25.0
        a = max(0, u - PAD)
        b = min(L, u + PAD + 1)
        row[a:b] -= 1.0 / (b - a)
        for s in np.nonzero(row)[0]:
            if s < 24:
                Econ[s, up] = row[s]
            elif s >= L - 24:
                Econ[64 + s - (L - 24), up] = row[s]
            else:
                raise AssertionError((up, s))

    rev = np.zeros((64, 64), np.float32)
    for i in range(64):
        rev[i, 63 - i] = 1.0
    return phi48, ident, MgT.astype(np.float32), Econ.astype(np.float32), rev


def _build():
    import concourse.bass as bass
    import concourse.tile as tile
    import concourse.mybir as mybir
    from concourse import bacc
    import bass_rust
    import ml_dtypes

    dt = mybir.dt
    f32 = dt.float32
    bf16 = dt.bfloat16
    AF = mybir.ActivationFunctionType
    ALU = mybir.AluOpType
    AX = mybir.AxisListType
    ts = bass.ts

    nc = bacc.Bacc(None, target_bir_lowering=False)

    xe = nc.dram_tensor("xb", [L, D], f32, kind="ExternalInput")
    wqe = nc.dram_tensor("Wq", [D, D], f32, kind="ExternalInput")
    wke = nc.dram_tensor("Wk", [D, D], f32, kind="ExternalInput")
    wve = nc.dram_tensor("Wv", [D, D], f32, kind="ExternalInput")
    woe = nc.dram_tensor("Wo", [D, D], f32, kind="ExternalInput")
    bqe = nc.dram_tensor("bq", [D], f32, kind="ExternalInput")
    bke = nc.dram_tensor("bk", [D], f32, kind="ExternalInput")
    bve = nc.dram_tensor("bv", [D], f32, kind="ExternalInput")
    boe = nc.dram_tensor("bo", [D], f32, kind="ExternalInput")
    gme = nc.dram_tensor("gamma", [D], f32, kind="ExternalInput")
    oute = nc.dram_tensor("out", [L, D], f32, kind="ExternalOutput")

    phi48, ident, MgT, Econ, rev = _np_consts()
    bf = ml_dtypes.bfloat16
    cphi = nc.inline_tensor(phi48.astype(bf), "c_phi")
    cid = nc.inline_tensor(ident.astype(bf), "c_id")
    cMgT = nc.inline_tensor(MgT.astype(bf), "c_MgT")
    cEcon = nc.inline_tensor(Econ.astype(bf), "c_Econ")
    crev = nc.inline_tensor(rev.astype(bf), "c_rev")
    cones = nc.inline_tensor(np.ones((1, 128), bf), "c_o128b")
    cones64 = nc.inline_tensor(np.ones((1, 64), np.float32), "c_o64")

    from contextlib import ExitStack

    with tile.TileContext(nc) as tc, ExitStack() as ctx:
        pc = ctx.enter_context(tc.tile_pool(name="consts", bufs=1))
        px = ctx.enter_context(tc.tile_pool(name="xpairs", bufs=NP))
        pz = ctx.enter_context(tc.tile_pool(name="ztiles", bufs=10))
        pzT = ctx.enter_context(tc.tile_pool(name="zT", bufs=3))
        py = ctx.enter_context(tc.tile_pool(name="ytiles", bufs=NT))
        pwb = ctx.enter_context(tc.tile_pool(name="wbf", bufs=1))
        pst = ctx.enter_context(tc.tile_pool(name="stats", bufs=2))
        pbs = ctx.enter_context(tc.tile_pool(name="bnscr", bufs=2))
        psm = ctx.enter_context(tc.tile_pool(name="smalls", bufs=1))
        pout = ctx.enter_context(tc.tile_pool(name="osb", bufs=3))
        poutE = ctx.enter_context(tc.tile_pool(name="osbE", bufs=1))
        pdram = ctx.enter_context(tc.tile_pool(name="dram", bufs=1, space="DRAM"))
        q_y = ctx.enter_context(tc.tile_pool(name="ps_y", bufs=2, space="PSUM"))
        q_tp = ctx.enter_context(tc.tile_pool(name="ps_tp", bufs=1, space="PSUM"))
        q_big = ctx.enter_context(tc.tile_pool(name="ps_big", bufs=2, space="PSUM"))
        q_corr = ctx.enter_context(tc.tile_pool(name="ps_cr", bufs=1, space="PSUM"))
        q_sm = ctx.enter_context(tc.tile_pool(name="ps_sm", bufs=1, space="PSUM"))

        # ---------------- DRAM scratch ----------------
        canv_g = pdram.tile([128, 384], bf16, tag="canv_g")
        canv_c = pdram.tile([24, 192], bf16, tag="canv_c")
        g88d = pdram.tile([88], bf16, tag="g88d")
        wrevd = pdram.tile([64], bf16, tag="wrevd")
        ccin = pdram.tile([64], f32, tag="ccin")
        ccout = pdram.tile([8, 64], f32, tag="ccout")
        mvd = pdram.tile([64], f32, tag="mvd")

        # ---------------- stats path (high priority) ----------------
        with tc.high_priority():
            x48 = pc.tile([128, 512], f32, tag="x48")
            nc.sync.dma_start(x48[0:24, :], xe[0:24, :])
            nc.sync.dma_start(x48[64:88, :], xe[L - 24 : L, :])
            phis = pc.tile([128, 1], bf16, tag="phis")
            nc.sync.dma_start(phis[:], cphi[:, :])
            idt = pc.tile([128, 128], bf16, tag="idt")
            nc.sync.dma_start(idt[:], cid[:, :])
            gammaP = pc.tile([128, 4], f32, tag="gammaP")
            nc.sync.dma_start(gammaP[:], gme[:].rearrange("(a b) -> b a", b=128))
            bqv = pc.tile([1, 512], f32, tag="bqv")
            nc.sync.dma_start(bqv[:], bqe[:])
            bkv = pc.tile([1, 512], f32, tag="bkv")
            nc.sync.dma_start(bkv[:], bke[:])

            # Wq/Wk interleaved bf16 via casting gpsimd DMAs:
            # wqk[:, c*1024 .. +512] = Wq chunk c; +512.. = Wk chunk c
            wqk = pwb.tile([128, 4096], bf16, tag="wqk")
            nc.gpsimd.dma_start(
                wqk[:].rearrange("p (c e d) -> p c e d", c=4, e=2)[:, :, 0, :],
                wqe[:, :].rearrange("(c p) d -> p c d", c=4),
            )
            nc.gpsimd.dma_start(
                wqk[:].rearrange("p (c e d) -> p c e d", c=4, e=2)[:, :, 1, :],
                wke[:, :].rearrange("(c p) d -> p c d", c=4),
            )

            # LN for the 48 boundary rows via bn_stats
            z48 = pc.tile([128, 512], bf16, tag="z48")
            nc.vector.memset(z48[:, :], 0.0)
            bs48 = pbs.tile([128, 8], f32, tag="bs48")
            nc.vector.bn_stats(bs48[0:88, 0:6], x48[0:88, :])
            nc.vector.bn_aggr(bs48[0:88, 6:8], bs48[0:88, 0:6])
            st8 = pst.tile([128, 4], f32, tag="st48")
            nc.vector.tensor_scalar(
                st8[0:88, 0:1], bs48[0:88, 7:8], 1.0, EPS, op0=ALU.mult, op1=ALU.add
            )
            nc.scalar.activation(st8[0:88, 1:2], st8[0:88, 0:1], AF.Sqrt)
            nc.vector.reciprocal(st8[0:88, 2:3], st8[0:88, 1:2])
            nc.vector.tensor_tensor(
                st8[0:88, 3:4], bs48[0:88, 6:7], st8[0:88, 2:3], op=ALU.mult
            )
            nc.vector.tensor_scalar(
                st8[0:88, 3:4], st8[0:88, 3:4], -1.0, None, op0=ALU.mult
            )
            nc.scalar.activation(
                z48[0:24, :], x48[0:24, :], AF.Identity,
                bias=st8[0:24, 3:4], scale=st8[0:24, 2:3],
            )
            nc.scalar.activation(
                z48[64:88, :], x48[64:88, :], AF.Identity,
                bias=st8[64:88, 3:4], scale=st8[64:88, 2:3],
            )
            qps = q_sm.tile([128, 8], f32, tag="sm")
            for c in range(4):
                nc.tensor.matmul(
                    qps[:, c : c + 1], z48[0:88, ts(c, 128)], phis[0:88, :],
                    start=True, stop=True,
                )

            bq_sc = pc.tile([1, 512], f32, tag="bq_sc")
            nc.scalar.mul(bq_sc[:], bqv[:], float(L))
            bk_sc = pc.tile([1, 512], f32, tag="bk_sc")
            nc.scalar.mul(bk_sc[:], bkv[:], float(L))

            qs_g = psm.tile([128, 4], bf16, tag="qsg")
            gb = psm.tile([128, 4], bf16, tag="gb")
            nc.vector.tensor_copy(gb[:], gammaP[:])
            nc.vector.tensor_tensor(qs_g[:], qps[:, 0:4], gb[:], op=ALU.mult)

            qq_ps = q_sm.tile([1, 512], f32, tag="sm")
            for c in range(4):
                nc.tensor.matmul(
                    qq_ps[:], qs_g[:, c : c + 1], wqk[:, c * 1024 : c * 1024 + 512],
                    start=(c == 0), stop=(c == 3),
                )
            qsv = psm.tile([1, 512], f32, tag="qsv")
            nc.vector.tensor_tensor(qsv[:], qq_ps[:], bq_sc[:], op=ALU.add)
            kk_ps = q_sm.tile([1, 512], f32, tag="sm")
            for c in range(4):
                nc.tensor.matmul(
                    kk_ps[:], qs_g[:, c : c + 1],
                    wqk[:, c * 1024 + 512 : (c + 1) * 1024],
                    start=(c == 0), stop=(c == 3),
                )
            ksv = psm.tile([1, 512], f32, tag="ksv")
            nc.vector.tensor_tensor(ksv[:], kk_ps[:], bk_sc[:], op=ALU.add)

            pr = psm.tile([1, 512], f32, tag="pr")
            nc.vector.tensor_tensor(pr[:], qsv[:], ksv[:], op=ALU.mult)
            mvr = psm.tile([1, 64], f32, tag="mvr")
            nc.vector.tensor_reduce(
                mvr[:], pr[:].rearrange("p (h c) -> p c h", h=H),
                axis=AX.X, op=ALU.add,
            )
            mv = psm.tile([1, 64], f32, tag="mv")
            nc.scalar.mul(mv[:], mvr[:], 1.0 / HL)
            nc.gpsimd.dma_start(ccin[:], mv[:])
            nc.gpsimd.dma_start(mvd[:], mv[:])

        # ---------------- weights (casting DMAs on gpsimd) ----------------
        wvb = pwb.tile([128, 2048], bf16, tag="wvb")
        nc.gpsimd.dma_start(
            wvb[:].rearrange("p (a d) -> p a d", a=4),
            wve[:, :].rearrange("(a p) d -> p a d", a=4),
        )
        wob = pwb.tile([128, 2048], bf16, tag="wob")
        nc.gpsimd.dma_start(
            wob[:].rearrange("p (a d) -> p a d", a=4),
            woe[:, :].rearrange("(a p) d -> p a d", a=4),
        )

        # zero toeplitz canvases
        zline = pc.tile([128, 384], bf16, tag="zline")
        nc.vector.memset(zline[:], 0.0)
        nc.gpsimd.dma_start(canv_g[:], zline[:])
        nc.gpsimd.dma_start(canv_c[:], zline[0:24, 0:192])

        # ---------------- x pair loads ----------------
        xp = [None] * NP

        def load_pair(k, eng):
            xt = px.tile([128, 1024], f32, tag="xp")
            eng.dma_start(
                xt[:].rearrange("p (c d) -> p c d", c=2),
                xe[256 * k : 256 * (k + 1), :].rearrange("(c p) d -> p c d", c=2),
            )
            xp[k] = xt

        for k in range(6):
            load_pair(k, nc.sync)
        for k in range(6, 12):
            load_pair(k, nc.scalar)

        bvP = pc.tile([128, 4], f32, tag="bvP")
        nc.sync.dma_start(bvP[:], bve[:].rearrange("(a b) -> b a", b=128))
        bov = pc.tile([1, 512], f32, tag="bov")
        nc.sync.dma_start(bov[:], boe[:])
        o128 = pc.tile([1, 128], bf16, tag="o128")
        nc.sync.dma_start(o128[:], cones[:, :])
        MgTsb = pc.tile([64, 88], bf16, tag="MgTsb")
        nc.sync.dma_start(MgTsb[:], cMgT[:, :])
        Econsb = pc.tile([128, 24], bf16, tag="Econsb")
        nc.sync.dma_start(Econsb[:], cEcon[:, :])
        revsb = pc.tile([64, 64], bf16, tag="revsb")
        nc.sync.dma_start(revsb[:], crev[:, :])
        o1x64 = pc.tile([1, 64], f32, tag="o1x64")
        nc.sync.dma_start(o1x64[:], cones64[:, :])

        # ---------------- weight prep: Wvo = diag(gamma) Wv Wo ----------------
        wvs = []
        for a in range(4):
            w = pwb.tile([128, 512], bf16, tag=f"wvs{a}")
            nc.scalar.activation(
                w[:], wvb[:, ts(a, 512)], AF.Identity, scale=gammaP[:, a : a + 1]
            )
            wvs.append(w)
        wvT = []
        for c in range(4):
            w = pwb.tile([128, 512], bf16, tag=f"wvT{c}")
            wvT.append(w)
        for a in range(4):
            tp = q_tp.tile([128, 512], bf16, tag="tp")
            for c in range(4):
                nc.tensor.transpose(tp[:, ts(c, 128)], wvs[a][:, ts(c, 128)], idt[:])
            for c in range(4):
                nc.vector.tensor_copy(wvT[c][:, ts(a, 128)], tp[:, ts(c, 128)])
        wvo = []
        for a in range(4):
            vps = q_y.tile([128, 512], f32, tag="y")
            for c in range(4):
                nc.tensor.matmul(
                    vps[:], wvT[c][:, ts(a, 128)], wob[:, ts(c, 512)],
                    start=(c == 0), stop=(c == 3),
                )
            w = pwb.tile([128, 512], bf16, tag=f"wvo{a}")
            nc.scalar.copy(w[:], vps[:])
            wvo.append(w)

        # cvb = bv @ Wo + bo, broadcast to [128, 512]
        bvPb = psm.tile([128, 4], bf16, tag="bvPb")
        nc.vector.tensor_copy(bvPb[:], bvP[:])
        cps = q_sm.tile([1, 512], f32, tag="sm")
        for c in range(4):
            nc.tensor.matmul(
                cps[:], bvPb[:, c : c + 1], wob[:, ts(c, 512)],
                start=(c == 0), stop=(c == 3),
            )
        cv_sb = psm.tile([1, 512], f32, tag="cv")
        nc.vector.tensor_tensor(cv_sb[:], cps[:], bov[:], op=ALU.add)
        cvb = psm.tile([1, 512], bf16, tag="cvb")
        nc.vector.tensor_copy(cvb[:], cv_sb[:])
        cbps = q_big.tile([128, 512], f32, tag="big")
        nc.tensor.matmul(cbps[:], o128[:], cvb[:], start=True, stop=True)
        cvbtile = pc.tile([128, 512], f32, tag="cvbtile")
        nc.vector.tensor_copy(cvbtile[:], cbps[:])

        # ---------------- main pipeline: LN -> z -> zT(PE) -> y ----------------
        ytiles = [None] * NT
        ztiles = [None] * NT

        def emit_group(g):  # tiles 4g .. 4g+3 (pairs 2g, 2g+1)
            st = pst.tile([128, 16], f32, tag="st")
            for j in range(4):
                i = 4 * g + j
                xh = xp[i // 2][:, ts(i % 2, 512)]
                bs = pbs.tile([128, 8], f32, tag="bs")
                nc.vector.bn_stats(bs[:, 0:6], xh)
                nc.vector.bn_aggr(st[:, 2 * j : 2 * j + 2], bs[:, 0:6])
            nc.vector.tensor_scalar(
                st[:, 8:12], st[:, 1:8:2], 1.0, EPS, op0=ALU.mult, op1=ALU.add
            )
            nc.scalar.activation(st[:, 12:16], st[:, 8:12], AF.Sqrt)
            nc.vector.reciprocal(st[:, 8:12], st[:, 12:16])
            nc.vector.tensor_tensor(
                st[:, 12:16], st[:, 0:8:2], st[:, 8:12], op=ALU.mult
            )
            nc.vector.tensor_scalar(
                st[:, 12:16], st[:, 12:16], -1.0, None, op0=ALU.mult
            )
            for j in range(4):
                i = 4 * g + j
                z = pz.tile([128, 512], bf16, tag="z")
                nc.scalar.activation(
                    z[:], xp[i // 2][:, ts(i % 2, 512)], AF.Identity,
                    bias=st[:, 12 + j : 13 + j], scale=st[:, 8 + j : 9 + j],
                )
                ztiles[i] = z

        def emit_y(i):
            tp = q_tp.tile([128, 512], bf16, tag="tp")
            for c in range(4):
                nc.tensor.transpose(tp[:, ts(c, 128)], ztiles[i][:, ts(c, 128)], idt[:])
            sT = pzT.tile([128, 512], bf16, tag="sT")
            nc.vector.tensor_copy(sT[:], tp[:])
            vps = q_y.tile([128, 512], f32, tag="y")
            for c in range(4):
                nc.tensor.matmul(
                    vps[:], sT[:, ts(c, 128)], wvo[c][:],
                    start=(c == 0), stop=(c == 3),
                )
            yv = py.tile([128, 512], bf16, tag="y")
            if i % 2 == 0:
                nc.scalar.copy(yv[:], vps[:])
            else:
                nc.vector.tensor_copy(yv[:], vps[:])
            ytiles[i] = yv

        for g in range(6):
            emit_group(g)
            for j in range(4):
                emit_y(4 * g + j)

        # collective launch: emitted after all pipeline DRAM-reading DMAs so
        # none of them pick up a false dependency on the AllGather; the gpsimd
        # queue still reaches this instruction within ~15us.
        nc.gpsimd.collective_compute(
            "AllGather",
            ALU.bypass,
            replica_groups=[list(range(NCORES))],
            ins=[ccin[:].opt()],
            outs=[ccout[:].opt()],
        )

        # residual with cvb folded in: xp[k] += cvbtile (both halves, vector)
        for k in range(NP):
            for h in range(2):
                nc.vector.tensor_tensor(
                    xp[k][:, ts(h, 512)], xp[k][:, ts(h, 512)], cvbtile[:],
                    op=ALU.add,
                )

        # ---------------- boundary ey = E(y) ----------------
        ytail = pc.tile([128, 512], bf16, tag="ytail")
        nc.gpsimd.dma_start(ytail[64:88, :], ytiles[23][104:128, :])
        eyps = q_big.tile([128, 512], f32, tag="big")
        nc.tensor.matmul(
            eyps[0:24, :], Econsb[0:24, :], ytiles[0][0:24, :],
            start=True, stop=False,
        )
        nc.tensor.matmul(
            eyps[0:24, :], Econsb[64:88, :], ytail[64:88, :],
            start=False, stop=True,
        )
        eysb = psm.tile([24, 512], bf16, tag="eysb")
        nc.vector.tensor_copy(eysb[:], eyps[0:24, :])

        # ---------------- collective readback -> weights -> toeplitz ----------------
        g8 = psm.tile([1, 512], f32, tag="g8")
        nc.sync.dma_start(g8[:], ccout[:])
        mvP = psm.tile([64, 1], f32, tag="mvP")
        nc.scalar.dma_start(mvP[:], mvd[:])
        sc = psm.tile([64, 8], f32, tag="scm")
        nc.scalar.activation(sc[:, 2:3], mvP[:], AF.Exp)  # local, pre-collective

        g_row = psm.tile([1, 64], f32, tag="grow")
        nc.vector.tensor_reduce(
            g_row[:], g8[:].rearrange("p (r c) -> p c r", r=NCORES),
            axis=AX.X, op=ALU.add,
        )
        ones64 = nc.const_aps.tensor(1.0, (64, 1))
        gP_ps = q_sm.tile([64, 1], f32, tag="sm")
        nc.tensor.matmul(
            gP_ps[:], g_row[:], nc.const_aps.tensor(1.0, (1, 1)),
            start=True, stop=True,
        )
        gP = psm.tile([64, 1], f32, tag="gP")
        nc.vector.tensor_copy(gP[:], gP_ps[:])
        gf_ps = q_sm.tile([64, 64], f32, tag="sm")
        nc.tensor.matmul(gf_ps[:], o1x64[:], g_row[:], start=True, stop=True)
        cmp = psm.tile([64, 64], f32, tag="cmp")
        nc.vector.tensor_tensor(
            cmp[:], gf_ps[:], gP[:].to_broadcast((64, 64)), op=ALU.is_gt
        )
        nc.vector.tensor_reduce(sc[:, 0:1], cmp[:], axis=AX.X, op=ALU.add)
        nc.vector.tensor_scalar(
            sc[:, 1:2], sc[:, 0:1], KTOP - 0.5, None, op0=ALU.is_lt
        )
        nc.vector.tensor_tensor(sc[:, 3:4], sc[:, 2:3], sc[:, 1:2], op=ALU.mult)
        s_ps = q_sm.tile([1, 1], f32, tag="sm")
        nc.tensor.matmul(s_ps[:], sc[:, 3:4], ones64, start=True, stop=True)
        rs = psm.tile([1, 1], f32, tag="rs")
        nc.vector.reciprocal(rs[:], s_ps[:])
        rsf_ps = q_sm.tile([64, 1], f32, tag="sm")
        nc.tensor.matmul(rsf_ps[:], o1x64[:], rs[:], start=True, stop=True)
        wf = psm.tile([64, 1], f32, tag="wf")
        nc.vector.tensor_tensor(wf[:], sc[:, 3:4], rsf_ps[:], op=ALU.mult)
        wfb = psm.tile([64, 1], bf16, tag="wfb")
        nc.vector.tensor_copy(wfb[:], wf[:])

        VP = bass_rust.VecI64Pair

        # g-kernel path (sync queue): g -> DRAM -> scatter -> canvas -> lhsTs
        g_ps = q_sm.tile([88, 1], f32, tag="sm")
        nc.tensor.matmul(g_ps[:], MgTsb[:], wfb[:], start=True, stop=True)
        g88b = psm.tile([88, 1], bf16, tag="g88b")
        nc.vector.tensor_copy(g88b[:], g_ps[:])
        nc.sync.dma_start(g88d[:], g88b[:])
        dstG = canv_g[:].flatten()
        dstG.ap = VP([[385, 128], [1, 88]])
        srcG = g88d[:].flatten()
        srcG.ap = VP([[0, 128], [1, 88]])
        nc.sync.dma_start(dstG, srcG)
        dstA = canv_g[0:1, 332:333].flatten()
        dstA.ap = VP([[385, 12], [1, 12]])
        srcA = g88d[:].flatten()
        srcA.ap = VP([[0, 12], [1, 12]])
        nc.sync.dma_start(dstA, srcA)
        canvsb = pc.tile([128, 384], bf16, tag="canvsb")
        nc.sync.dma_start(canvsb[:], canv_g[:])
        toep = {}
        for name, c0 in (("M", 12), ("N", 140), ("A", 216)):
            tpp = q_tp.tile([128, 512], bf16, tag="tp")
            nc.tensor.transpose(tpp[:, 0:128], canvsb[:, c0 : c0 + 128], idt[:])
            t_sb = pc.tile([128, 128], bf16, tag=f"toep{name}")
            nc.vector.tensor_copy(t_sb[:], tpp[:, 0:128])
            toep[name] = t_sb

        # corr path (scalar queue): wrev -> DRAM -> scatter -> corr lhsT
        wrev_ps = q_sm.tile([64, 1], f32, tag="sm")
        nc.tensor.matmul(wrev_ps[:], revsb[:], wfb[:], start=True, stop=True)
        wrevb = psm.tile([64, 1], bf16, tag="wrevb")
        nc.vector.tensor_copy(wrevb[:], wrev_ps[:])
        nc.scalar.dma_start(wrevd[:], wrevb[:])
        dstH = canv_c[0:1, 65:66].flatten()
        dstH.ap = VP([[193, 12], [1, 64]])
        srcH = wrevd[:].flatten()
        srcH.ap = VP([[0, 12], [1, 64]])
        nc.scalar.dma_start(dstH, srcH)
        dstT = canv_c[12:13, 53:54].flatten()
        dstT.ap = VP([[193, 12], [1, 64]])
        srcT = wrevd[:].flatten()
        srcT.ap = VP([[0, 12], [1, 64]])
        nc.scalar.dma_start(dstT, srcT)
        cvsb = pc.tile([24, 192], bf16, tag="cvsb")
        nc.scalar.dma_start(cvsb[:], canv_c[:])
        nc.vector.tensor_tensor(
            cvsb[:, 0:64], cvsb[:, 0:64], cvsb[:, 128:192], op=ALU.add
        )
        corrP = q_corr.tile([128, 512], f32, tag="cr")
        nc.tensor.matmul(
            corrP[:], cvsb[:, 0:128], eysb[:], start=True, stop=True
        )

        # ---------------- taps + residual + output ----------------
        # order: 1..22 first (corr-free), then 0 and 23 (need corrP)
        obs = {}

        def emit_tap(i):
            tps = q_big.tile([128, 512], f32, tag="big")
            nc.tensor.matmul(
                tps[:], toep["A"][:], ytiles[(i + NT - 1) % NT][:],
                start=True, stop=False,
            )
            nc.tensor.matmul(
                tps[:], toep["M"][:], ytiles[i][:], start=False, stop=False
            )
            nc.tensor.matmul(
                tps[:], toep["N"][:], ytiles[(i + 1) % NT][:],
                start=False, stop=True,
            )
            k, hh = i // 2, i % 2
            if k not in obs:
                if k in (0, NP - 1):
                    obs[k] = poutE.tile([128, 1024], f32, tag=f"obE{k}", name="obE")
                else:
                    obs[k] = pout.tile([128, 1024], f32, tag="ob", name="ob")
            ob = obs[k]
            nc.vector.tensor_tensor(
                ob[:, ts(hh, 512)], xp[k][:, ts(hh, 512)], tps[:], op=ALU.add
            )
            if i == 0:
                nc.vector.tensor_tensor(
                    ob[0:32, 0:512], ob[0:32, 0:512], corrP[0:32, :], op=ALU.add
                )
            if i == 23:
                nc.vector.tensor_tensor(
                    ob[32:64, 512:1024], ob[32:64, 512:1024], corrP[32:64, :],
                    op=ALU.add,
                )
                nc.vector.tensor_tensor(
                    ob[64:128, 512:1024], ob[64:128, 512:1024], corrP[64:128, :],
                    op=ALU.add,
                )

        def flush_pair(k, eng):
            eng.dma_start(
                oute[256 * k : 256 * (k + 1), :].rearrange(
                    "(c p) d -> p c d", c=2
                ),
                obs[k][:].rearrange("p (c d) -> p c d", c=2),
            )

        for i in range(1, 23):
            emit_tap(i)
            if i % 2 == 1 and i >= 3:  # pair i//2 in 1..10 complete
                flush_pair(i // 2, nc.gpsimd)
        emit_tap(0)
        emit_tap(23)
        flush_pair(0, nc.gpsimd)
        flush_pair(11, nc.gpsimd)

    nc.finalize()
    return nc


def _get_nc():
    if "nc" not in _CACHE:
        _CACHE["nc"] = _build()
    return _CACHE["nc"]


def kernel_ext(inputs, trace=False):
    from concourse.bass_utils import run_bass_kernel_spmd

    nc = _get_nc()
    x = np.ascontiguousarray(inputs["x"], np.float32)
    common = {
        k: np.ascontiguousarray(inputs[k], np.float32)
        for k in ["Wq", "Wk", "Wv", "Wo", "bq", "bk", "bv", "bo", "gamma"]
    }
    in_maps = [{"xb": x[i], **common} for i in range(NCORES)]
    res = run_bass_kernel_spmd(nc, in_maps, list(range(NCORES)), trace=trace)
    out = np.stack([res.results[i]["out"] for i in range(NCORES)], axis=0)
    return out, res


def kernel(**inputs):
    out, _ = kernel_ext(inputs)
    return out
